# revision 51
# baseline (speedup 1.0000x reference)
"""Collective-free causal attention: scores = x(Wq^T Wk)x^T, out = (P x)Wv^T.

Core c = (batch c//2, query-stripe h = c%2); stripe h owns interleaved
128-row query tiles g = 2t + (1-h), t in 0..8, which balances the causal
triangle across the pair without any cross-core communication.

Device math (all matmul inputs bf16, accumulation f32 in PSUM):
  M    = Wq^T Wk * scale            (host, weight-only preprocessing)
  A^T  = M^T x_q^T                  [e', q]    phase A
  S^T  = x^T(stripes) . A^T         [s, q]     per (t, s-tile), N=128
  P^T  = exp(S^T) (* tri-mask on the 1-2 diagonal tiles, data-driven)
  r    = P^T^T @ ones               rowsums via PE, PSUM-accumulated
  Z^T  = x . P^T                    [e, q]     PSUM-accumulated over s
  out  = (Z^T^T @ Wv^T) * (1/r)     [q, f]
The [s,q] layout means exp output feeds the PV/rowsum matmuls directly as
the stationary operand - no PE transposes anywhere.
"""

import numpy as np

B, S, E, KD = 4, 2048, 1024, 1024
NCORES = 8
P = 128
NQT = 8          # own query tiles per core
NST = 16         # 128-row key tiles per batch
SCALE = 1.0 / float(np.sqrt(KD))

PIPE = 3         # score->exp->PV software pipeline depth (in st-steps)

_prog_cache = {}


def _build_body(ctx, tc, ap):
    from concourse import mybir

    nc = tc.nc
    f32 = mybir.dt.float32
    bf16 = mybir.dt.bfloat16
    Exp = mybir.ActivationFunctionType.Exp
    Copy = mybir.ActivationFunctionType.Copy

    # ---- persistent SBUF inputs (packed [128, k*cols] fold layouts)
    wp = ctx.enter_context(tc.tile_pool(name="wp", bufs=1))
    m_sb = wp.tile([P, 8 * 1024], bf16, name="m_sb")
    xtq_sb = wp.tile([P, 8 * 1024], bf16, name="xtq_sb")
    xts_sb = wp.tile([P, 8 * 2048], bf16, name="xts_sb")
    xn_sb = wp.tile([P, 16 * 1024], bf16, name="xn_sb")
    wvt_sb = wp.tile([P, 8 * 1024], bf16, name="wvt_sb")
    at_sb = wp.tile([P, 8 * 1024], bf16, name="at_sb")
    maskp = wp.tile([P, P], bf16, name="maskp")
    maskl = wp.tile([P, P], bf16, name="maskl")
    # one wide all-ones tile serves both the rowsum vector (col 0) and the
    # PE-warmup operands: a single memset halves the startup chain
    ones_w = wp.tile([P, 512], bf16, name="ones_w")
    nc.vector.memset(ones_w, 1.0)
    ones = ones_w[:, 0:1]
    scratch = ones_w

    # ---- input DMAs, ordered by first use
    for et in range(8):
        nc.sync.dma_start(out=m_sb[:, et * 1024:(et + 1) * 1024],
                          in_=ap["m"][:, et * 1024:(et + 1) * 1024])
        nc.sync.dma_start(out=xtq_sb[:, et * 1024:(et + 1) * 1024],
                          in_=ap["xtq"][:, et * 1024:(et + 1) * 1024])
    nc.sync.dma_start(out=maskp, in_=ap["maskp"])
    nc.sync.dma_start(out=maskl, in_=ap["maskl"])
    for c in range(16):
        nc.sync.dma_start(out=xts_sb[:, c * 1024:(c + 1) * 1024],
                          in_=ap["xts"][:, c * 1024:(c + 1) * 1024])
    for c in range(2):
        nc.sync.dma_start(out=xn_sb[:, c * 1024:(c + 1) * 1024],
                          in_=ap["xn"][:, c * 1024:(c + 1) * 1024])
    for ez in range(8):
        nc.sync.dma_start(out=wvt_sb[:, ez * 1024:(ez + 1) * 1024],
                          in_=ap["wvt"][:, ez * 1024:(ez + 1) * 1024])
    for c in range(2, 16):
        nc.sync.dma_start(out=xn_sb[:, c * 1024:(c + 1) * 1024],
                          in_=ap["xn"][:, c * 1024:(c + 1) * 1024])

    # ---- PSUM pools, allocated up-front and shared by both phases
    # (banks: sp 2 + zt 4 + rs 1 + op 1 = 8; no pool handoff barrier)
    sp = ctx.enter_context(tc.tile_pool(name="sp", bufs=1, space="PSUM"))
    ztp = ctx.enter_context(tc.tile_pool(name="ztp", bufs=2, space="PSUM"))
    rsp = ctx.enter_context(tc.tile_pool(name="rsp", bufs=1, space="PSUM"))
    opp = ctx.enter_context(tc.tile_pool(name="opp", bufs=1, space="PSUM"))
    ptp = ctx.enter_context(tc.tile_pool(name="ptp", bufs=PIPE + 3))
    ztsb = ctx.enter_context(tc.tile_pool(name="ztsb", bufs=2))
    rvp = ctx.enter_context(tc.tile_pool(name="rvp", bufs=2))
    osp = ctx.enter_context(tc.tile_pool(name="osp", bufs=3))

    rsfull = rsp.tile([P, 512], f32, name="rsfull")

    # ---- phase A: A^T[e',q] = sum_e M[e,e'] xTq[e,q], computed on phase B's
    # own PSUM tiles, ordered so each bank is evicted (freed) in the same
    # order phase B will claim it (sp first, op last).
    pa = [sp.tile([P, 512], f32, name="pas0", tag="sp0"),
          sp.tile([P, 512], f32, name="pas1", tag="sp1"),
          ztp.tile([P, 512], f32, name="paz0", tag="zt0"),
          ztp.tile([P, 512], f32, name="paz1", tag="zt1"),
          rsfull,
          ztp.tile([P, 512], f32, name="paz2", tag="zt0"),
          ztp.tile([P, 512], f32, name="paz3", tag="zt1"),
          opp.tile([P, 512], f32, name="pao", tag="op")]

    # PE warmup on garbage SBUF (no input deps): keeps the PE busy through
    # the initial DMA latency so the pstate ramp completes before the first
    # real matmul and phase A runs at full clock from the start.
    # warmup burns the ~3us pstate ramp on dummy work during the DMA head,
    # ending right as the first (m, xtq) chunk lands
    for _ in range(7):
        nc.tensor.matmul(pa[7], scratch[:, 0:P], scratch,
                         start=True, stop=True, skip_group_check=True)

    def pa_mm(ep, et, hf):
        nc.tensor.matmul(
            pa[ep],
            m_sb[:, et * 1024 + ep * P: et * 1024 + (ep + 1) * P],
            xtq_sb[:, et * 1024 + hf * 512: et * 1024 + (hf + 1) * 512],
            start=(et == 0), stop=(et == 7))

    def pa_evict(ep, hf):
        dst = at_sb[:, ep * 1024 + hf * 512: ep * 1024 + (hf + 1) * 512]
        if ep == 7 and hf == 1:
            # last eviction: split so both engines clear before phase B's
            # exp/mask chain needs them
            nc.scalar.copy(dst[:, 0:256], pa[ep][:, 0:256])
            nc.vector.tensor_copy(dst[:, 256:512], pa[ep][:, 256:512])
        elif ep % 2 == 0:
            nc.scalar.copy(dst, pa[ep])
        else:
            nc.vector.tensor_copy(dst, pa[ep])

    # hf0: et-major, consumes (m, xtq) DMA chunk pairs in arrival order
    for et in range(8):
        for ep in range(8):
            pa_mm(ep, et, 0)
            if et == 7:
                pa_evict(ep, 0)
    # hf1: ep-major (all inputs resident by now) so group stops stagger
    # 1.7us apart and evictions never backlog ACT/DVE into phase B
    for ep in range(8):
        for et in range(8):
            pa_mm(ep, et, 1)
        pa_evict(ep, 1)

    out_t = ap["out"].rearrange("(t p) f -> t p f", p=P)

    steps = [(t, st) for t in range(NQT) for st in range(2 * t + 2)]
    # Rowsum accumulator: zeroed by DVE (not matmul start=True) so the bank's
    # zero-region WAR chain stays off the PE timeline at tile boundaries.
    rs = rsfull[:, 0:1]
    nc.vector.memset(rs, 0.0)
    state = {}   # t -> zt tiles ([128,512] x2, 4 e-slices each)
    zts_of = {}  # t -> evicted SBUF zt tiles
    rinv_of = {}
    # Two score banks, alternating per step: a matmul group's start=True
    # write-locks its whole 2KB zero region, so consecutive steps must use
    # different banks or each step serializes on the previous step's exp read.
    spcur = {}   # parity -> rolling [128,512] tile, 4 st-slices

    def emit_scores(i, t, st):
        g = 2 * t + 1
        par, n = i % 2, i // 2
        if n % 4 == 0:
            spcur[par] = sp.tile([P, 512], f32, name=f"sps{par}",
                                 tag=f"sp{par}")
        ps = spcur[par][:, (n % 4) * P:(n % 4 + 1) * P]
        for ep in range(8):
            nc.tensor.matmul(
                ps,
                xts_sb[:, ep * 2048 + st * P: ep * 2048 + (st + 1) * P],
                at_sb[:, ep * 1024 + t * P: ep * 1024 + (t + 1) * P],
                start=(ep == 0), stop=(ep == 7))
        pt = ptp.tile([P, P], bf16, name="pt", tag="pt")
        nc.scalar.activation(pt, ps, Exp)
        # masks run on the otherwise-idle GPSIMD engine, off the ACT/DVE
        # critical chain
        if st == g - 1:
            nc.gpsimd.tensor_mul(pt, pt, maskp)
        elif st == g:
            nc.gpsimd.tensor_mul(pt, pt, maskl)
        return pt

    def emit_op(t, hf, alt_bank=False, strips=1):
        zs = zts_of[t]
        rv = rinv_of[t]
        if alt_bank:  # final OP: spare zt slot, avoids WAR on the op bank
            po = ztp.tile([P, 512], f32, name="po2", tag="zt0")
        else:
            po = opp.tile([P, 512], f32, name="po", tag="op")
        for ez in range(8):
            nc.tensor.matmul(
                po, zs[ez // 4][:, (ez % 4) * P:(ez % 4 + 1) * P],
                wvt_sb[:, ez * 1024 + hf * 512: ez * 1024 + (hf + 1) * 512],
                start=(ez == 0), stop=(ez == 7))
        w = 512 // strips
        for s in range(strips):
            ob = osp.tile([P, w], bf16, name="ob", tag=f"ob{s}")
            nc.scalar.activation(ob, po[:, s * w:(s + 1) * w], Copy, scale=rv)
            nc.sync.dma_start(
                out=out_t[t][:, hf * 512 + s * w: hf * 512 + (s + 1) * w],
                in_=ob)

    def emit_rz(t, st, pt):
        g = 2 * t + 1
        if st == 0:
            state[t] = [ztp.tile([P, 512], f32, name=f"zt{j}", tag=f"zt{j}")
                        for j in range(2)]
        zt = state[t]
        nc.tensor.matmul(rs, pt, ones, start=False, stop=(st == g),
                         skip_group_check=True)
        for ez in range(8):
            # one accumulation group per zt tile: start/stop only on that
            # tile's first/last matmul of the whole st loop (2KB zero region)
            nc.tensor.matmul(
                zt[ez // 4][:, (ez % 4) * P:(ez % 4 + 1) * P],
                xn_sb[:, st * 1024 + ez * P: st * 1024 + (ez + 1) * P],
                pt,
                start=(st == 0 and ez % 4 == 0),
                stop=(st == g and ez % 4 == 3))
        if st == g:
            zs = []
            nw = 4 if t == NQT - 1 else 2  # finer strips at the tail
            for j in range(2):
                # strip across both engines: OP(t) can start ~500ns after
                # the last RZ instead of waiting a full 512-col copy
                z = ztsb.tile([P, 512], bf16, name=f"zs{j}", tag=f"zs{j}")
                for s in range(nw):
                    w0, w1 = s * 512 // nw, (s + 1) * 512 // nw
                    if s % 2 == 1:  # first strip on DVE: shorter chain to OP
                        nc.scalar.copy(z[:, w0:w1], zt[j][:, w0:w1])
                    else:
                        nc.vector.tensor_copy(z[:, w0:w1], zt[j][:, w0:w1])
                zs.append(z)
            zts_of[t] = zs
            rv = rvp.tile([P, 1], f32, name="rv", tag="rv")
            nc.vector.reciprocal(rv, rs)
            nc.vector.memset(rs, 0.0)
            rinv_of[t] = rv
            del state[t]
        elif t > 0 and st == 0:
            emit_op(t - 1, 0)
        elif t > 0 and st == 2:
            emit_op(t - 1, 1)

    pend = []
    for i in range(len(steps) + PIPE):
        if i < len(steps):
            t, st = steps[i]
            pend.append((t, st, emit_scores(i, t, st)))
        if i >= PIPE:
            t, st, pt = pend.pop(0)
            emit_rz(t, st, pt)
    emit_op(NQT - 1, 0)
    emit_op(NQT - 1, 1, alt_bank=True)


def build_program():
    if "nc" in _prog_cache:
        return _prog_cache["nc"]
    from contextlib import ExitStack
    from concourse import bacc, mybir
    import concourse.tile as tile

    nc = bacc.Bacc("TRN2", target_bir_lowering=False, debug=False,
                   num_devices=NCORES)
    f32 = mybir.dt.float32
    bf16 = mybir.dt.bfloat16
    ap = {
        "m": nc.dram_tensor("m", [P, 8 * 1024], bf16, kind="ExternalInput").ap(),
        "xtq": nc.dram_tensor("xtq", [P, 8 * 1024], bf16, kind="ExternalInput").ap(),
        "xts": nc.dram_tensor("xts", [P, 16 * 1024], bf16, kind="ExternalInput").ap(),
        "xn": nc.dram_tensor("xn", [P, 16 * 1024], bf16, kind="ExternalInput").ap(),
        "wvt": nc.dram_tensor("wvt", [P, 8 * 1024], bf16, kind="ExternalInput").ap(),
        "maskp": nc.dram_tensor("maskp", [P, P], bf16, kind="ExternalInput").ap(),
        "maskl": nc.dram_tensor("maskl", [P, P], bf16, kind="ExternalInput").ap(),
        "out": nc.dram_tensor("out", [1024, E], bf16, kind="ExternalOutput").ap(),
    }
    with tile.TileContext(nc) as tc:
        with ExitStack() as ctx:
            _build_body(ctx, tc, ap)
    nc.compile()
    _prog_cache["nc"] = nc
    return nc


def _fold(a, nt, cols):
    # [nt*128, cols] -> [128, nt*cols] with block j at cols [j*cols:(j+1)*cols]
    return np.ascontiguousarray(
        a.reshape(nt, P, cols).transpose(1, 0, 2).reshape(P, nt * cols))


def make_in_maps(x, W_q, W_k, W_v):
    import ml_dtypes
    bf = ml_dtypes.bfloat16
    x = np.asarray(x, np.float32)
    W_q = np.asarray(W_q, np.float32)
    W_k = np.asarray(W_k, np.float32)
    W_v = np.asarray(W_v, np.float32)

    M = (W_q.T @ W_k) * SCALE                      # [e, e'], scale folded
    m_p = _fold(M, 8, 1024).astype(bf)
    wvt_p = _fold(np.ascontiguousarray(W_v.T), 8, 1024).astype(bf)

    i = np.arange(P)[:, None]
    j = np.arange(P)[None, :]
    tri = (i <= j).astype(np.float32)              # allow s_local <= q_local
    masks = [(np.ones((P, P), np.float32), tri),   # h=0: odd tiles, diag last
             (tri, np.zeros((P, P), np.float32))]  # h=1: even tiles

    in_maps = []
    for c in range(NCORES):
        b, h = c // 2, c % 2
        xb = x[b]                                  # [2048, 1024]
        xT = np.ascontiguousarray(xb.T)            # [1024, 2048]
        qcols = np.concatenate(
            [np.arange((2 * t + 1 - h) * P, (2 * t + 2 - h) * P)
             for t in range(NQT)])
        xq = np.ascontiguousarray(xb[qcols].T)     # [1024 e, 1024 q]
        mp, ml = masks[h]
        in_maps.append({
            "m": m_p,
            "xtq": _fold(xq, 8, 1024).astype(bf),
            "xts": _fold(xT, 8, 2048).astype(bf),
            "xn": _fold(xb, 16, 1024).astype(bf),
            "wvt": wvt_p,
            "maskp": mp.astype(bf),
            "maskl": ml.astype(bf),
        })
    return in_maps


def assemble(results):
    out = np.zeros((B, S, E), np.float32)
    for c in range(NCORES):
        b, h = c // 2, c % 2
        co = results[c]["out"]
        for t in range(NQT):
            g = 2 * t + (1 - h)
            out[b, g * P:(g + 1) * P, :] = co[t * P:(t + 1) * P]
    return out


def kernel(x, W_q, W_k, W_v):
    from concourse.bass_utils import run_bass_kernel_spmd
    nc = build_program()
    in_maps = make_in_maps(x, W_q, W_k, W_v)
    res = run_bass_kernel_spmd(nc, in_maps, core_ids=list(range(NCORES)))
    return assemble(res.results)


# revision 53
# speedup vs baseline: 1.0016x; 1.0016x over previous
"""Collective-free causal attention: scores = x(Wq^T Wk)x^T, out = (P x)Wv^T.

Core c = (batch c//2, query-stripe h = c%2); stripe h owns interleaved
128-row query tiles g = 2t + (1-h), t in 0..8, which balances the causal
triangle across the pair without any cross-core communication.

Device math (all matmul inputs bf16, accumulation f32 in PSUM):
  M    = Wq^T Wk * scale            (host, weight-only preprocessing)
  A^T  = M^T x_q^T                  [e', q]    phase A
  S^T  = x^T(stripes) . A^T         [s, q]     per (t, s-tile), N=128
  P^T  = exp(S^T) (* tri-mask on the 1-2 diagonal tiles, data-driven)
  r    = P^T^T @ ones               rowsums via PE, PSUM-accumulated
  Z^T  = x . P^T                    [e, q]     PSUM-accumulated over s
  out  = (Z^T^T @ Wv^T) * (1/r)     [q, f]
The [s,q] layout means exp output feeds the PV/rowsum matmuls directly as
the stationary operand - no PE transposes anywhere.
"""

import numpy as np

B, S, E, KD = 4, 2048, 1024, 1024
NCORES = 8
P = 128
NQT = 8          # own query tiles per core
NST = 16         # 128-row key tiles per batch
SCALE = 1.0 / float(np.sqrt(KD))

PIPE = 3         # score->exp->PV software pipeline depth (in st-steps)

_prog_cache = {}


def _build_body(ctx, tc, ap):
    from concourse import mybir

    nc = tc.nc
    f32 = mybir.dt.float32
    bf16 = mybir.dt.bfloat16
    Exp = mybir.ActivationFunctionType.Exp
    Copy = mybir.ActivationFunctionType.Copy

    # ---- persistent SBUF inputs (packed [128, k*cols] fold layouts)
    wp = ctx.enter_context(tc.tile_pool(name="wp", bufs=1))
    m_sb = wp.tile([P, 8 * 1024], bf16, name="m_sb")
    xtq_sb = wp.tile([P, 8 * 1024], bf16, name="xtq_sb")
    xts_sb = wp.tile([P, 8 * 2048], bf16, name="xts_sb")
    xn_sb = wp.tile([P, 16 * 1024], bf16, name="xn_sb")
    wvt_sb = wp.tile([P, 8 * 1024], bf16, name="wvt_sb")
    at_sb = wp.tile([P, 8 * 1024], bf16, name="at_sb")
    maskp = wp.tile([P, P], bf16, name="maskp")
    maskl = wp.tile([P, P], bf16, name="maskl")
    ones = wp.tile([P, 1], bf16, name="ones")
    scratch = wp.tile([P, 512], bf16, name="scratch")  # warmup fodder
    nc.vector.memset(scratch, 0.0)
    nc.vector.memset(ones, 1.0)

    # ---- input DMAs, ordered by first use
    for et in range(8):
        nc.sync.dma_start(out=m_sb[:, et * 1024:(et + 1) * 1024],
                          in_=ap["m"][:, et * 1024:(et + 1) * 1024])
        nc.sync.dma_start(out=xtq_sb[:, et * 1024:(et + 1) * 1024],
                          in_=ap["xtq"][:, et * 1024:(et + 1) * 1024])
    nc.sync.dma_start(out=maskp, in_=ap["maskp"])
    nc.sync.dma_start(out=maskl, in_=ap["maskl"])
    for c in range(16):
        nc.sync.dma_start(out=xts_sb[:, c * 1024:(c + 1) * 1024],
                          in_=ap["xts"][:, c * 1024:(c + 1) * 1024])
    for c in range(2):
        nc.sync.dma_start(out=xn_sb[:, c * 1024:(c + 1) * 1024],
                          in_=ap["xn"][:, c * 1024:(c + 1) * 1024])
    for ez in range(8):
        nc.sync.dma_start(out=wvt_sb[:, ez * 1024:(ez + 1) * 1024],
                          in_=ap["wvt"][:, ez * 1024:(ez + 1) * 1024])
    for c in range(2, 16):
        nc.sync.dma_start(out=xn_sb[:, c * 1024:(c + 1) * 1024],
                          in_=ap["xn"][:, c * 1024:(c + 1) * 1024])

    # ---- PSUM pools, allocated up-front and shared by both phases
    # (banks: sp 2 + zt 4 + rs 1 + op 1 = 8; no pool handoff barrier)
    sp = ctx.enter_context(tc.tile_pool(name="sp", bufs=1, space="PSUM"))
    ztp = ctx.enter_context(tc.tile_pool(name="ztp", bufs=2, space="PSUM"))
    rsp = ctx.enter_context(tc.tile_pool(name="rsp", bufs=1, space="PSUM"))
    opp = ctx.enter_context(tc.tile_pool(name="opp", bufs=1, space="PSUM"))
    ptp = ctx.enter_context(tc.tile_pool(name="ptp", bufs=PIPE + 3))
    ztsb = ctx.enter_context(tc.tile_pool(name="ztsb", bufs=2))
    rvp = ctx.enter_context(tc.tile_pool(name="rvp", bufs=2))
    osp = ctx.enter_context(tc.tile_pool(name="osp", bufs=3))

    rsfull = rsp.tile([P, 512], f32, name="rsfull")

    # ---- phase A: A^T[e',q] = sum_e M[e,e'] xTq[e,q], computed on phase B's
    # own PSUM tiles, ordered so each bank is evicted (freed) in the same
    # order phase B will claim it (sp first, op last).
    pa = [sp.tile([P, 512], f32, name="pas0", tag="sp0"),
          sp.tile([P, 512], f32, name="pas1", tag="sp1"),
          ztp.tile([P, 512], f32, name="paz0", tag="zt0"),
          ztp.tile([P, 512], f32, name="paz1", tag="zt1"),
          rsfull,
          ztp.tile([P, 512], f32, name="paz2", tag="zt0"),
          ztp.tile([P, 512], f32, name="paz3", tag="zt1"),
          opp.tile([P, 512], f32, name="pao", tag="op")]

    # PE warmup on garbage SBUF (no input deps): keeps the PE busy through
    # the initial DMA latency so the pstate ramp completes before the first
    # real matmul and phase A runs at full clock from the start.
    # warmup burns the ~3us pstate ramp on dummy work during the DMA head,
    # ending right as the first (m, xtq) chunk lands
    for _ in range(7):
        nc.tensor.matmul(pa[7], scratch[:, 0:P], scratch,
                         start=True, stop=True, skip_group_check=True)

    def pa_mm(ep, et, hf):
        nc.tensor.matmul(
            pa[ep],
            m_sb[:, et * 1024 + ep * P: et * 1024 + (ep + 1) * P],
            xtq_sb[:, et * 1024 + hf * 512: et * 1024 + (hf + 1) * 512],
            start=(et == 0), stop=(et == 7))

    def pa_evict(ep, hf):
        dst = at_sb[:, ep * 1024 + hf * 512: ep * 1024 + (hf + 1) * 512]
        if ep == 7 and hf == 1:
            # last eviction: split so both engines clear before phase B's
            # exp/mask chain needs them
            nc.scalar.copy(dst[:, 0:256], pa[ep][:, 0:256])
            nc.vector.tensor_copy(dst[:, 256:512], pa[ep][:, 256:512])
        elif ep % 2 == 0:
            nc.scalar.copy(dst, pa[ep])
        else:
            nc.vector.tensor_copy(dst, pa[ep])

    # hf0: et-major, consumes (m, xtq) DMA chunk pairs in arrival order
    for et in range(8):
        for ep in range(8):
            pa_mm(ep, et, 0)
            if et == 7:
                pa_evict(ep, 0)
    # hf1: ep-major (all inputs resident by now) so group stops stagger
    # 1.7us apart and evictions never backlog ACT/DVE into phase B
    for ep in range(8):
        for et in range(8):
            pa_mm(ep, et, 1)
        pa_evict(ep, 1)

    out_t = ap["out"].rearrange("(t p) f -> t p f", p=P)

    steps = [(t, st) for t in range(NQT) for st in range(2 * t + 2)]
    # Rowsum accumulator: zeroed by DVE (not matmul start=True) so the bank's
    # zero-region WAR chain stays off the PE timeline at tile boundaries.
    rs = rsfull[:, 0:1]
    nc.vector.memset(rs, 0.0)
    state = {}   # t -> zt tiles ([128,512] x2, 4 e-slices each)
    zts_of = {}  # t -> evicted SBUF zt tiles
    rinv_of = {}
    # Two score banks, alternating per step: a matmul group's start=True
    # write-locks its whole 2KB zero region, so consecutive steps must use
    # different banks or each step serializes on the previous step's exp read.
    spcur = {}   # parity -> rolling [128,512] tile, 4 st-slices

    def emit_scores(i, t, st):
        g = 2 * t + 1
        par, n = i % 2, i // 2
        if n % 4 == 0:
            spcur[par] = sp.tile([P, 512], f32, name=f"sps{par}",
                                 tag=f"sp{par}")
        ps = spcur[par][:, (n % 4) * P:(n % 4 + 1) * P]
        for ep in range(8):
            nc.tensor.matmul(
                ps,
                xts_sb[:, ep * 2048 + st * P: ep * 2048 + (st + 1) * P],
                at_sb[:, ep * 1024 + t * P: ep * 1024 + (t + 1) * P],
                start=(ep == 0), stop=(ep == 7))
        pt = ptp.tile([P, P], bf16, name="pt", tag="pt")
        nc.scalar.activation(pt, ps, Exp)
        # masks run on the otherwise-idle GPSIMD engine, off the ACT/DVE
        # critical chain
        if st == g - 1:
            nc.gpsimd.tensor_mul(pt, pt, maskp)
        elif st == g:
            nc.gpsimd.tensor_mul(pt, pt, maskl)
        return pt

    def emit_op(t, hf, alt_bank=False, strips=1):
        zs = zts_of[t]
        rv = rinv_of[t]
        if alt_bank:  # final OP: spare zt slot, avoids WAR on the op bank
            po = ztp.tile([P, 512], f32, name="po2", tag="zt0")
        else:
            po = opp.tile([P, 512], f32, name="po", tag="op")
        for ez in range(8):
            nc.tensor.matmul(
                po, zs[ez // 4][:, (ez % 4) * P:(ez % 4 + 1) * P],
                wvt_sb[:, ez * 1024 + hf * 512: ez * 1024 + (hf + 1) * 512],
                start=(ez == 0), stop=(ez == 7))
        w = 512 // strips
        for s in range(strips):
            ob = osp.tile([P, w], bf16, name="ob", tag=f"ob{s}")
            nc.scalar.activation(ob, po[:, s * w:(s + 1) * w], Copy, scale=rv)
            nc.sync.dma_start(
                out=out_t[t][:, hf * 512 + s * w: hf * 512 + (s + 1) * w],
                in_=ob)

    def emit_rz(t, st, pt):
        g = 2 * t + 1
        if st == 0:
            state[t] = [ztp.tile([P, 512], f32, name=f"zt{j}", tag=f"zt{j}")
                        for j in range(2)]
        zt = state[t]
        nc.tensor.matmul(rs, pt, ones, start=False, stop=(st == g),
                         skip_group_check=True)
        for ez in range(8):
            # one accumulation group per zt tile: start/stop only on that
            # tile's first/last matmul of the whole st loop (2KB zero region)
            nc.tensor.matmul(
                zt[ez // 4][:, (ez % 4) * P:(ez % 4 + 1) * P],
                xn_sb[:, st * 1024 + ez * P: st * 1024 + (ez + 1) * P],
                pt,
                start=(st == 0 and ez % 4 == 0),
                stop=(st == g and ez % 4 == 3))
        if st == g:
            zs = []
            nw = 4 if t == NQT - 1 else 2  # finer strips at the tail
            for j in range(2):
                # strip across both engines: OP(t) can start ~500ns after
                # the last RZ instead of waiting a full 512-col copy
                z = ztsb.tile([P, 512], bf16, name=f"zs{j}", tag=f"zs{j}")
                for s in range(nw):
                    w0, w1 = s * 512 // nw, (s + 1) * 512 // nw
                    if s % 2 == 0:
                        nc.scalar.copy(z[:, w0:w1], zt[j][:, w0:w1])
                    else:
                        nc.vector.tensor_copy(z[:, w0:w1], zt[j][:, w0:w1])
                zs.append(z)
            zts_of[t] = zs
            rv = rvp.tile([P, 1], f32, name="rv", tag="rv")
            nc.vector.reciprocal(rv, rs)
            nc.vector.memset(rs, 0.0)
            rinv_of[t] = rv
            del state[t]
        elif t > 0 and st == 0:
            emit_op(t - 1, 0)
        elif t > 0 and st == 2:
            emit_op(t - 1, 1)

    pend = []
    for i in range(len(steps) + PIPE):
        if i < len(steps):
            t, st = steps[i]
            pend.append((t, st, emit_scores(i, t, st)))
        if i >= PIPE:
            t, st, pt = pend.pop(0)
            emit_rz(t, st, pt)
    emit_op(NQT - 1, 0)
    emit_op(NQT - 1, 1, alt_bank=True)


def build_program():
    if "nc" in _prog_cache:
        return _prog_cache["nc"]
    from contextlib import ExitStack
    from concourse import bacc, mybir
    import concourse.tile as tile

    nc = bacc.Bacc("TRN2", target_bir_lowering=False, debug=False,
                   num_devices=NCORES)
    f32 = mybir.dt.float32
    bf16 = mybir.dt.bfloat16
    ap = {
        "m": nc.dram_tensor("m", [P, 8 * 1024], bf16, kind="ExternalInput").ap(),
        "xtq": nc.dram_tensor("xtq", [P, 8 * 1024], bf16, kind="ExternalInput").ap(),
        "xts": nc.dram_tensor("xts", [P, 16 * 1024], bf16, kind="ExternalInput").ap(),
        "xn": nc.dram_tensor("xn", [P, 16 * 1024], bf16, kind="ExternalInput").ap(),
        "wvt": nc.dram_tensor("wvt", [P, 8 * 1024], bf16, kind="ExternalInput").ap(),
        "maskp": nc.dram_tensor("maskp", [P, P], bf16, kind="ExternalInput").ap(),
        "maskl": nc.dram_tensor("maskl", [P, P], bf16, kind="ExternalInput").ap(),
        "out": nc.dram_tensor("out", [1024, E], bf16, kind="ExternalOutput").ap(),
    }
    with tile.TileContext(nc) as tc:
        with ExitStack() as ctx:
            _build_body(ctx, tc, ap)
    nc.compile()
    _prog_cache["nc"] = nc
    return nc


def _fold(a, nt, cols):
    # [nt*128, cols] -> [128, nt*cols] with block j at cols [j*cols:(j+1)*cols]
    return np.ascontiguousarray(
        a.reshape(nt, P, cols).transpose(1, 0, 2).reshape(P, nt * cols))


def make_in_maps(x, W_q, W_k, W_v):
    import ml_dtypes
    bf = ml_dtypes.bfloat16
    x = np.asarray(x, np.float32)
    W_q = np.asarray(W_q, np.float32)
    W_k = np.asarray(W_k, np.float32)
    W_v = np.asarray(W_v, np.float32)

    M = (W_q.T @ W_k) * SCALE                      # [e, e'], scale folded
    m_p = _fold(M, 8, 1024).astype(bf)
    wvt_p = _fold(np.ascontiguousarray(W_v.T), 8, 1024).astype(bf)

    i = np.arange(P)[:, None]
    j = np.arange(P)[None, :]
    tri = (i <= j).astype(np.float32)              # allow s_local <= q_local
    masks = [(np.ones((P, P), np.float32), tri),   # h=0: odd tiles, diag last
             (tri, np.zeros((P, P), np.float32))]  # h=1: even tiles

    in_maps = []
    for c in range(NCORES):
        b, h = c // 2, c % 2
        xb = x[b]                                  # [2048, 1024]
        xT = np.ascontiguousarray(xb.T)            # [1024, 2048]
        qcols = np.concatenate(
            [np.arange((2 * t + 1 - h) * P, (2 * t + 2 - h) * P)
             for t in range(NQT)])
        xq = np.ascontiguousarray(xb[qcols].T)     # [1024 e, 1024 q]
        mp, ml = masks[h]
        in_maps.append({
            "m": m_p,
            "xtq": _fold(xq, 8, 1024).astype(bf),
            "xts": _fold(xT, 8, 2048).astype(bf),
            "xn": _fold(xb, 16, 1024).astype(bf),
            "wvt": wvt_p,
            "maskp": mp.astype(bf),
            "maskl": ml.astype(bf),
        })
    return in_maps


def assemble(results):
    out = np.zeros((B, S, E), np.float32)
    for c in range(NCORES):
        b, h = c // 2, c % 2
        co = results[c]["out"]
        for t in range(NQT):
            g = 2 * t + (1 - h)
            out[b, g * P:(g + 1) * P, :] = co[t * P:(t + 1) * P]
    return out


def kernel(x, W_q, W_k, W_v):
    from concourse.bass_utils import run_bass_kernel_spmd
    nc = build_program()
    in_maps = make_in_maps(x, W_q, W_k, W_v)
    res = run_bass_kernel_spmd(nc, in_maps, core_ids=list(range(NCORES)))
    return assemble(res.results)


# revision 54
# speedup vs baseline: 1.0020x; 1.0004x over previous
"""Collective-free causal attention: scores = x(Wq^T Wk)x^T, out = (P x)Wv^T.

Core c = (batch c//2, query-stripe h = c%2); stripe h owns interleaved
128-row query tiles g = 2t + (1-h), t in 0..8, which balances the causal
triangle across the pair without any cross-core communication.

Device math (all matmul inputs bf16, accumulation f32 in PSUM):
  M    = Wq^T Wk * scale            (host, weight-only preprocessing)
  A^T  = M^T x_q^T                  [e', q]    phase A
  S^T  = x^T(stripes) . A^T         [s, q]     per (t, s-tile), N=128
  P^T  = exp(S^T) (* tri-mask on the 1-2 diagonal tiles, data-driven)
  r    = P^T^T @ ones               rowsums via PE, PSUM-accumulated
  Z^T  = x . P^T                    [e, q]     PSUM-accumulated over s
  out  = (Z^T^T @ Wv^T) * (1/r)     [q, f]
The [s,q] layout means exp output feeds the PV/rowsum matmuls directly as
the stationary operand - no PE transposes anywhere.
"""

import numpy as np

B, S, E, KD = 4, 2048, 1024, 1024
NCORES = 8
P = 128
NQT = 8          # own query tiles per core
NST = 16         # 128-row key tiles per batch
SCALE = 1.0 / float(np.sqrt(KD))

PIPE = 3         # score->exp->PV software pipeline depth (in st-steps)

_prog_cache = {}


def _build_body(ctx, tc, ap):
    from concourse import mybir

    nc = tc.nc
    f32 = mybir.dt.float32
    bf16 = mybir.dt.bfloat16
    Exp = mybir.ActivationFunctionType.Exp
    Copy = mybir.ActivationFunctionType.Copy

    # ---- persistent SBUF inputs (packed [128, k*cols] fold layouts)
    wp = ctx.enter_context(tc.tile_pool(name="wp", bufs=1))
    m_sb = wp.tile([P, 8 * 1024], bf16, name="m_sb")
    xtq_sb = wp.tile([P, 8 * 1024], bf16, name="xtq_sb")
    xts_sb = wp.tile([P, 8 * 2048], bf16, name="xts_sb")
    xn_sb = wp.tile([P, 16 * 1024], bf16, name="xn_sb")
    wvt_sb = wp.tile([P, 8 * 1024], bf16, name="wvt_sb")
    at_sb = wp.tile([P, 8 * 1024], bf16, name="at_sb")
    maskp = wp.tile([P, P], bf16, name="maskp")
    maskl = wp.tile([P, P], bf16, name="maskl")
    ones = wp.tile([P, 1], bf16, name="ones")
    scratch = wp.tile([P, 512], bf16, name="scratch")  # warmup fodder
    nc.gpsimd.memset(scratch, 0.0)
    nc.vector.memset(ones, 1.0)

    # ---- input DMAs, ordered by first use
    for et in range(8):
        nc.sync.dma_start(out=m_sb[:, et * 1024:(et + 1) * 1024],
                          in_=ap["m"][:, et * 1024:(et + 1) * 1024])
        nc.sync.dma_start(out=xtq_sb[:, et * 1024:(et + 1) * 1024],
                          in_=ap["xtq"][:, et * 1024:(et + 1) * 1024])
    nc.sync.dma_start(out=maskp, in_=ap["maskp"])
    nc.sync.dma_start(out=maskl, in_=ap["maskl"])
    for c in range(16):
        nc.sync.dma_start(out=xts_sb[:, c * 1024:(c + 1) * 1024],
                          in_=ap["xts"][:, c * 1024:(c + 1) * 1024])
    for c in range(2):
        nc.sync.dma_start(out=xn_sb[:, c * 1024:(c + 1) * 1024],
                          in_=ap["xn"][:, c * 1024:(c + 1) * 1024])
    for ez in range(8):
        nc.sync.dma_start(out=wvt_sb[:, ez * 1024:(ez + 1) * 1024],
                          in_=ap["wvt"][:, ez * 1024:(ez + 1) * 1024])
    for c in range(2, 16):
        nc.sync.dma_start(out=xn_sb[:, c * 1024:(c + 1) * 1024],
                          in_=ap["xn"][:, c * 1024:(c + 1) * 1024])

    # ---- PSUM pools, allocated up-front and shared by both phases
    # (banks: sp 2 + zt 4 + rs 1 + op 1 = 8; no pool handoff barrier)
    sp = ctx.enter_context(tc.tile_pool(name="sp", bufs=1, space="PSUM"))
    ztp = ctx.enter_context(tc.tile_pool(name="ztp", bufs=2, space="PSUM"))
    rsp = ctx.enter_context(tc.tile_pool(name="rsp", bufs=1, space="PSUM"))
    opp = ctx.enter_context(tc.tile_pool(name="opp", bufs=1, space="PSUM"))
    ptp = ctx.enter_context(tc.tile_pool(name="ptp", bufs=PIPE + 3))
    ztsb = ctx.enter_context(tc.tile_pool(name="ztsb", bufs=2))
    rvp = ctx.enter_context(tc.tile_pool(name="rvp", bufs=2))
    osp = ctx.enter_context(tc.tile_pool(name="osp", bufs=3))

    rsfull = rsp.tile([P, 512], f32, name="rsfull")

    # ---- phase A: A^T[e',q] = sum_e M[e,e'] xTq[e,q], computed on phase B's
    # own PSUM tiles, ordered so each bank is evicted (freed) in the same
    # order phase B will claim it (sp first, op last).
    pa = [sp.tile([P, 512], f32, name="pas0", tag="sp0"),
          sp.tile([P, 512], f32, name="pas1", tag="sp1"),
          ztp.tile([P, 512], f32, name="paz0", tag="zt0"),
          ztp.tile([P, 512], f32, name="paz1", tag="zt1"),
          rsfull,
          ztp.tile([P, 512], f32, name="paz2", tag="zt0"),
          ztp.tile([P, 512], f32, name="paz3", tag="zt1"),
          opp.tile([P, 512], f32, name="pao", tag="op")]

    # PE warmup on garbage SBUF (no input deps): keeps the PE busy through
    # the initial DMA latency so the pstate ramp completes before the first
    # real matmul and phase A runs at full clock from the start.
    # warmup burns the ~3us pstate ramp on dummy work during the DMA head,
    # ending right as the first (m, xtq) chunk lands
    for _ in range(7):
        nc.tensor.matmul(pa[7], scratch[:, 0:P], scratch,
                         start=True, stop=True, skip_group_check=True)

    def pa_mm(ep, et, hf):
        nc.tensor.matmul(
            pa[ep],
            m_sb[:, et * 1024 + ep * P: et * 1024 + (ep + 1) * P],
            xtq_sb[:, et * 1024 + hf * 512: et * 1024 + (hf + 1) * 512],
            start=(et == 0), stop=(et == 7))

    def pa_evict(ep, hf):
        dst = at_sb[:, ep * 1024 + hf * 512: ep * 1024 + (hf + 1) * 512]
        if ep == 7 and hf == 1:
            # last eviction: split so both engines clear before phase B's
            # exp/mask chain needs them
            nc.scalar.copy(dst[:, 0:256], pa[ep][:, 0:256])
            nc.vector.tensor_copy(dst[:, 256:512], pa[ep][:, 256:512])
        elif ep % 2 == 0:
            nc.scalar.copy(dst, pa[ep])
        else:
            nc.vector.tensor_copy(dst, pa[ep])

    # hf0: et-major, consumes (m, xtq) DMA chunk pairs in arrival order
    for et in range(8):
        for ep in range(8):
            pa_mm(ep, et, 0)
            if et == 7:
                pa_evict(ep, 0)
    # hf1: ep-major (all inputs resident by now) so group stops stagger
    # 1.7us apart and evictions never backlog ACT/DVE into phase B
    for ep in range(8):
        for et in range(8):
            pa_mm(ep, et, 1)
        pa_evict(ep, 1)

    out_t = ap["out"].rearrange("(t p) f -> t p f", p=P)

    steps = [(t, st) for t in range(NQT) for st in range(2 * t + 2)]
    # Rowsum accumulator: zeroed by DVE (not matmul start=True) so the bank's
    # zero-region WAR chain stays off the PE timeline at tile boundaries.
    rs = rsfull[:, 0:1]
    nc.vector.memset(rs, 0.0)
    state = {}   # t -> zt tiles ([128,512] x2, 4 e-slices each)
    zts_of = {}  # t -> evicted SBUF zt tiles
    rinv_of = {}
    # Two score banks, alternating per step: a matmul group's start=True
    # write-locks its whole 2KB zero region, so consecutive steps must use
    # different banks or each step serializes on the previous step's exp read.
    spcur = {}   # parity -> rolling [128,512] tile, 4 st-slices

    def emit_scores(i, t, st):
        g = 2 * t + 1
        par, n = i % 2, i // 2
        if n % 4 == 0:
            spcur[par] = sp.tile([P, 512], f32, name=f"sps{par}",
                                 tag=f"sp{par}")
        ps = spcur[par][:, (n % 4) * P:(n % 4 + 1) * P]
        for ep in range(8):
            nc.tensor.matmul(
                ps,
                xts_sb[:, ep * 2048 + st * P: ep * 2048 + (st + 1) * P],
                at_sb[:, ep * 1024 + t * P: ep * 1024 + (t + 1) * P],
                start=(ep == 0), stop=(ep == 7))
        pt = ptp.tile([P, P], bf16, name="pt", tag="pt")
        nc.scalar.activation(pt, ps, Exp)
        # masks run on the otherwise-idle GPSIMD engine, off the ACT/DVE
        # critical chain
        if st == g - 1:
            nc.gpsimd.tensor_mul(pt, pt, maskp)
        elif st == g:
            nc.gpsimd.tensor_mul(pt, pt, maskl)
        return pt

    def emit_op(t, hf, alt_bank=False, strips=1):
        zs = zts_of[t]
        rv = rinv_of[t]
        if alt_bank:  # final OP: spare zt slot, avoids WAR on the op bank
            po = ztp.tile([P, 512], f32, name="po2", tag="zt0")
        else:
            po = opp.tile([P, 512], f32, name="po", tag="op")
        for ez in range(8):
            nc.tensor.matmul(
                po, zs[ez // 4][:, (ez % 4) * P:(ez % 4 + 1) * P],
                wvt_sb[:, ez * 1024 + hf * 512: ez * 1024 + (hf + 1) * 512],
                start=(ez == 0), stop=(ez == 7))
        w = 512 // strips
        for s in range(strips):
            ob = osp.tile([P, w], bf16, name="ob", tag=f"ob{s}")
            nc.scalar.activation(ob, po[:, s * w:(s + 1) * w], Copy, scale=rv)
            nc.sync.dma_start(
                out=out_t[t][:, hf * 512 + s * w: hf * 512 + (s + 1) * w],
                in_=ob)

    def emit_rz(t, st, pt):
        g = 2 * t + 1
        if st == 0:
            state[t] = [ztp.tile([P, 512], f32, name=f"zt{j}", tag=f"zt{j}")
                        for j in range(2)]
        zt = state[t]
        nc.tensor.matmul(rs, pt, ones, start=False, stop=(st == g),
                         skip_group_check=True)
        for ez in range(8):
            # one accumulation group per zt tile: start/stop only on that
            # tile's first/last matmul of the whole st loop (2KB zero region)
            nc.tensor.matmul(
                zt[ez // 4][:, (ez % 4) * P:(ez % 4 + 1) * P],
                xn_sb[:, st * 1024 + ez * P: st * 1024 + (ez + 1) * P],
                pt,
                start=(st == 0 and ez % 4 == 0),
                stop=(st == g and ez % 4 == 3))
        if st == g:
            zs = []
            nw = 4 if t == NQT - 1 else 2  # finer strips at the tail
            for j in range(2):
                # strip across both engines: OP(t) can start ~500ns after
                # the last RZ instead of waiting a full 512-col copy
                z = ztsb.tile([P, 512], bf16, name=f"zs{j}", tag=f"zs{j}")
                for s in range(nw):
                    w0, w1 = s * 512 // nw, (s + 1) * 512 // nw
                    if s % 2 == 0:
                        nc.scalar.copy(z[:, w0:w1], zt[j][:, w0:w1])
                    else:
                        nc.vector.tensor_copy(z[:, w0:w1], zt[j][:, w0:w1])
                zs.append(z)
            zts_of[t] = zs
            rv = rvp.tile([P, 1], f32, name="rv", tag="rv")
            nc.vector.reciprocal(rv, rs)
            nc.vector.memset(rs, 0.0)
            rinv_of[t] = rv
            del state[t]
        elif t > 0 and st == 0:
            emit_op(t - 1, 0)
        elif t > 0 and st == 2:
            emit_op(t - 1, 1)

    pend = []
    for i in range(len(steps) + PIPE):
        if i < len(steps):
            t, st = steps[i]
            pend.append((t, st, emit_scores(i, t, st)))
        if i >= PIPE:
            t, st, pt = pend.pop(0)
            emit_rz(t, st, pt)
    emit_op(NQT - 1, 0)
    emit_op(NQT - 1, 1, alt_bank=True)


def build_program():
    if "nc" in _prog_cache:
        return _prog_cache["nc"]
    from contextlib import ExitStack
    from concourse import bacc, mybir
    import concourse.tile as tile

    nc = bacc.Bacc("TRN2", target_bir_lowering=False, debug=False,
                   num_devices=NCORES)
    f32 = mybir.dt.float32
    bf16 = mybir.dt.bfloat16
    ap = {
        "m": nc.dram_tensor("m", [P, 8 * 1024], bf16, kind="ExternalInput").ap(),
        "xtq": nc.dram_tensor("xtq", [P, 8 * 1024], bf16, kind="ExternalInput").ap(),
        "xts": nc.dram_tensor("xts", [P, 16 * 1024], bf16, kind="ExternalInput").ap(),
        "xn": nc.dram_tensor("xn", [P, 16 * 1024], bf16, kind="ExternalInput").ap(),
        "wvt": nc.dram_tensor("wvt", [P, 8 * 1024], bf16, kind="ExternalInput").ap(),
        "maskp": nc.dram_tensor("maskp", [P, P], bf16, kind="ExternalInput").ap(),
        "maskl": nc.dram_tensor("maskl", [P, P], bf16, kind="ExternalInput").ap(),
        "out": nc.dram_tensor("out", [1024, E], bf16, kind="ExternalOutput").ap(),
    }
    with tile.TileContext(nc) as tc:
        with ExitStack() as ctx:
            _build_body(ctx, tc, ap)
    nc.compile()
    _prog_cache["nc"] = nc
    return nc


def _fold(a, nt, cols):
    # [nt*128, cols] -> [128, nt*cols] with block j at cols [j*cols:(j+1)*cols]
    return np.ascontiguousarray(
        a.reshape(nt, P, cols).transpose(1, 0, 2).reshape(P, nt * cols))


def make_in_maps(x, W_q, W_k, W_v):
    import ml_dtypes
    bf = ml_dtypes.bfloat16
    x = np.asarray(x, np.float32)
    W_q = np.asarray(W_q, np.float32)
    W_k = np.asarray(W_k, np.float32)
    W_v = np.asarray(W_v, np.float32)

    M = (W_q.T @ W_k) * SCALE                      # [e, e'], scale folded
    m_p = _fold(M, 8, 1024).astype(bf)
    wvt_p = _fold(np.ascontiguousarray(W_v.T), 8, 1024).astype(bf)

    i = np.arange(P)[:, None]
    j = np.arange(P)[None, :]
    tri = (i <= j).astype(np.float32)              # allow s_local <= q_local
    masks = [(np.ones((P, P), np.float32), tri),   # h=0: odd tiles, diag last
             (tri, np.zeros((P, P), np.float32))]  # h=1: even tiles

    in_maps = []
    for c in range(NCORES):
        b, h = c // 2, c % 2
        xb = x[b]                                  # [2048, 1024]
        xT = np.ascontiguousarray(xb.T)            # [1024, 2048]
        qcols = np.concatenate(
            [np.arange((2 * t + 1 - h) * P, (2 * t + 2 - h) * P)
             for t in range(NQT)])
        xq = np.ascontiguousarray(xb[qcols].T)     # [1024 e, 1024 q]
        mp, ml = masks[h]
        in_maps.append({
            "m": m_p,
            "xtq": _fold(xq, 8, 1024).astype(bf),
            "xts": _fold(xT, 8, 2048).astype(bf),
            "xn": _fold(xb, 16, 1024).astype(bf),
            "wvt": wvt_p,
            "maskp": mp.astype(bf),
            "maskl": ml.astype(bf),
        })
    return in_maps


def assemble(results):
    out = np.zeros((B, S, E), np.float32)
    for c in range(NCORES):
        b, h = c // 2, c % 2
        co = results[c]["out"]
        for t in range(NQT):
            g = 2 * t + (1 - h)
            out[b, g * P:(g + 1) * P, :] = co[t * P:(t + 1) * P]
    return out


def kernel(x, W_q, W_k, W_v):
    from concourse.bass_utils import run_bass_kernel_spmd
    nc = build_program()
    in_maps = make_in_maps(x, W_q, W_k, W_v)
    res = run_bass_kernel_spmd(nc, in_maps, core_ids=list(range(NCORES)))
    return assemble(res.results)
